# revision 8
# baseline (speedup 1.0000x reference)
"""Trainium2 Bass kernel for per-expert MLP (MoE experts, expert-parallel).

Computes out = relu(relu(x @ w1) @ w2) per expert.
  x:  [E=32, N=1024, D_IN=3072] f32
  w1: [E, D_IN, D_H=1024] f32
  w2: [E, D_H, D_OUT=256] f32
  out:[E, N, D_OUT] f32

Sharding: expert dim E=32 split across 8 cores (4 experts/core), no
communication. Host pre-casts and pre-tiles layouts so every DMA is a
plain partition-major copy and no on-chip transposes are needed.

Precision scheme (error budget rel_l2 < 2e-2):
  - GEMM1 K dim (3072 = 24 d-tiles of 128) split: first NF8=6 d-tiles
    use fp8 e4m3 operands via DoubleRow matmuls (2 k-tiles per matmul
    at 2x rate), remaining 18 in fp16 (same PE cost as bf16, 8x less
    quantization noise). Measured rel_l2 ~1.9e-2, dominated by the fp8
    tiles (per-pair err 1.096e-2, sqrt(3)*that total).

Compute scheme:
  - GEMM1 computes hiddenT (h on partitions): lhsT = w1 [d,h] tile,
    rhs = xT [d,n] tile.
  - Experts 1-3: the fp16 portion (K=2304) runs one level of Strassen
    (2x2 over d/h/n halves): 7 products of [1152k x 512h x 512n]
    instead of 8 - 63 matmuls per output quadrant-row vs 72. Operand
    sums on the vector engine (fp16), psum combines on the vector
    engine into SBUF f32 accumulators, relu on scalar. The fp8 portion
    accumulates into per-quadrant psums and joins in the combine.
    PSUM ring (ps1, 6 bufs) alloc order per mi:
    F8a,M7,M1,M4,M5,F8b,M3,F8c,M2,F8d,M6 - verified free-before-reuse.
  - Expert 0 runs the direct path: its ramp is DMA-bound, so compute
    must consume tiles at arrival rate; Strassen needs all operands
    before any product, which would idle the PE ~15us.
  - GEMM2 (K=1024, fp16) direct, computed transposed (psum [o,n]).
"""

import numpy as np
import ml_dtypes

E, N, D_IN, D_H, D_OUT = 32, 1024, 3072, 1024, 256
NCORES = 8
E_PER = E // NCORES  # 4 experts per core
P = 128
DT = D_IN // P  # 24 k-tiles for GEMM1
NF8 = 6         # leading k-tiles in fp8 e4m3 (even: DoubleRow pairs)
NPR = NF8 // 2  # DoubleRow pairs
DBF = DT - NF8  # 18 fp16 k-tiles
DH = DBF // 2   # 9 = d-tiles per Strassen half
HT = D_H // P   # 8 h-tiles
FD = 512        # matmul free dim (one PSUM bank of f32)
NCH = N // FD   # 2 n-chunks in GEMM1

_F16 = np.float16
_F8 = ml_dtypes.float8_e4m3
_CACHE = {}


def _build_program():
    """Build + compile the per-core Bass program (same program on all cores)."""
    if "nc" in _CACHE:
        return _CACHE["nc"], _CACHE["names"]

    from contextlib import ExitStack

    import concourse.bass as bass
    import concourse.tile as tile
    from concourse import bacc, mybir

    f16 = mybir.dt.float16
    f8 = mybir.dt.float8e4
    f32 = mybir.dt.float32
    DR = mybir.MatmulPerfMode.DoubleRow
    ADD = mybir.AluOpType.add
    SUB = mybir.AluOpType.subtract
    MULT = mybir.AluOpType.mult

    nc = bacc.Bacc("TRN2", target_bir_lowering=False, debug=False,
                   enable_asserts=False)

    x8_d = nc.dram_tensor("x8t", [E_PER, P, NF8, N], f8,
                          kind="ExternalInput").ap()
    # x fp16 split in d-halves (Strassen block operands)
    xa_d = nc.dram_tensor("xat", [E_PER, P, DH, N], f16,
                          kind="ExternalInput").ap()
    xb_d = nc.dram_tensor("xbt", [E_PER, P, DH, N], f16,
                          kind="ExternalInput").ap()
    w18_d = nc.dram_tensor("w18t", [E_PER, HT, P, NF8 * P], f8,
                           kind="ExternalInput").ap()
    w1_d = nc.dram_tensor("w1t", [E_PER, HT, P, DBF * P], f16,
                          kind="ExternalInput").ap()
    w2_d = nc.dram_tensor("w2t", [E_PER, P, HT, D_OUT], f16,
                          kind="ExternalInput").ap()
    out_d = nc.dram_tensor("out", [E_PER, D_OUT, N], f32,
                           kind="ExternalOutput").ap()

    relu = mybir.ActivationFunctionType.Relu
    HP = DH * P  # 1152: fp16 free-dim half of a w1 tile

    with tile.TileContext(nc) as tc, ExitStack() as ctx:
        xp8 = ctx.enter_context(tc.tile_pool(name="x8", bufs=2))
        xp = ctx.enter_context(tc.tile_pool(name="x", bufs=3))
        w1p8 = ctx.enter_context(tc.tile_pool(name="w18", bufs=8))
        w1p = ctx.enter_context(tc.tile_pool(name="w1", bufs=4))
        w2p = ctx.enter_context(tc.tile_pool(name="w2", bufs=2))
        hp = ctx.enter_context(tc.tile_pool(name="hid", bufs=1))
        op = ctx.enter_context(tc.tile_pool(name="o", bufs=1))
        wsp = ctx.enter_context(tc.tile_pool(name="ws", bufs=10))
        xsp = ctx.enter_context(tc.tile_pool(name="xs", bufs=5))
        ctp = ctx.enter_context(tc.tile_pool(name="ct", bufs=6))
        wmp = ctx.enter_context(tc.tile_pool(name="warm", bufs=1))
        ps1 = ctx.enter_context(tc.tile_pool(name="ps1", bufs=6, space="PSUM"))
        ps2 = ctx.enter_context(tc.tile_pool(name="ps2", bufs=2, space="PSUM"))

        def stt(out, in0, in1, op1):
            nc.vector.scalar_tensor_tensor(out, in0, 1.0, in1, MULT, op1)

        # PE warm-up: dummy matmuls with no data deps fill the initial DMA
        # wait so the HAM clock-gate is at 8/8 (2.4 GHz) when real matmuls
        # start (the un-throttle needs ~3.4us of sustained PE activity).
        NWARM = 18
        warm = wmp.tile([P, FD], f16, tag="warm")
        nc.vector.memset(warm[:], 0.0)
        pw = ps2.tile([P, FD], f32, tag="ps2", name="pw")
        for i in range(NWARM):
            nc.tensor.matmul(pw[:], warm[:, 0:P], warm[:],
                             start=(i == 0), stop=(i == NWARM - 1))

        for e in range(E_PER):
            w18_tiles = [None] * HT
            w1_tiles = [None] * HT
            x8_sb = xp8.tile([P, NF8, N], f8, tag="x8")
            xa_sb = xp.tile([P, DH, N], f16, tag="x")
            xb_sb = xp.tile([P, DH, N], f16, tag="x")

            def xd(d, ns=slice(0, N)):  # fp16 x d-tile view, d in 0..17
                return (xa_sb[:, d, ns] if d < DH else xb_sb[:, d - DH, ns])

            if e == 0:
                # Pace the first-expert ramp (DMA-bandwidth-bound): fp8
                # operands lead (half bytes, feed the leading DoubleRow
                # matmuls), w1 fp16 h0/h1 chunked so the first fp16
                # matmuls only wait on a ~192KB transfer.
                C6 = 6 * P
                w18a = w1p8.tile([P, NF8, P], f8, tag="w18")
                w18b = w1p8.tile([P, NF8, P], f8, tag="w18")
                w18_tiles[0], w18_tiles[1] = w18a, w18b
                w1_sb = w1p.tile([P, DBF * P], f16, tag="w1")
                w1b_sb = w1p.tile([P, DBF * P], f16, tag="w1")
                w1_tiles[0], w1_tiles[1] = w1_sb, w1b_sb
                nc.sync.dma_start(w18a[:], w18_d[e, 0])
                nc.sync.dma_start(w18b[:], w18_d[e, 1])
                nc.sync.dma_start(x8_sb[:, 0:2, :], x8_d[e, :, 0:2, :])
                nc.sync.dma_start(x8_sb[:, 2:4, :], x8_d[e, :, 2:4, :])
                nc.sync.dma_start(x8_sb[:, 4:6, :], x8_d[e, :, 4:6, :])
                nc.sync.dma_start(w1_sb[:, 0:C6], w1_d[e, 0, :, 0:C6])
                nc.sync.dma_start(xa_sb[:, 0, :], xa_d[e, :, 0, :])
                nc.sync.dma_start(w1b_sb[:, 0:C6], w1_d[e, 1, :, 0:C6])
                nc.sync.dma_start(xa_sb[:, 1, :], xa_d[e, :, 1, :])
                nc.sync.dma_start(xa_sb[:, 2, :], xa_d[e, :, 2, :])
                nc.sync.dma_start(w1_sb[:, C6: 2 * C6], w1_d[e, 0, :, C6: 2 * C6])
                nc.sync.dma_start(w1b_sb[:, C6: 2 * C6], w1_d[e, 1, :, C6: 2 * C6])
                nc.sync.dma_start(xa_sb[:, 3, :], xa_d[e, :, 3, :])
                nc.sync.dma_start(xa_sb[:, 4, :], xa_d[e, :, 4, :])
                nc.sync.dma_start(w1_sb[:, 2 * C6: DBF * P],
                                  w1_d[e, 0, :, 2 * C6: DBF * P])
                nc.sync.dma_start(w1b_sb[:, 2 * C6: DBF * P],
                                  w1_d[e, 1, :, 2 * C6: DBF * P])
                for d in range(5, DH):
                    nc.sync.dma_start(xa_sb[:, d, :], xa_d[e, :, d, :])
                for d in range(DH):
                    nc.sync.dma_start(xb_sb[:, d, :], xb_d[e, :, d, :])
                for h in range(2, HT):
                    w18_sb = w1p8.tile([P, NF8, P], f8, tag="w18")
                    nc.sync.dma_start(w18_sb[:], w18_d[e, h])
                    w18_tiles[h] = w18_sb
                    w1_sb = w1p.tile([P, DBF * P], f16, tag="w1")
                    nc.sync.dma_start(w1_sb[:], w1_d[e, h])
                    w1_tiles[h] = w1_sb
            else:
                # prefetched during previous expert; w1 tiles in paired
                # (h, 4+h) order - the Strassen mi loop consumes them in
                # pairs, and the 4-buf ring frees in the same order.
                for h in (0, 4):
                    w18_sb = w1p8.tile([P, NF8, P], f8, tag="w18")
                    nc.sync.dma_start(w18_sb[:], w18_d[e, h])
                    w18_tiles[h] = w18_sb
                    w1_sb = w1p.tile([P, DBF * P], f16, tag="w1")
                    nc.sync.dma_start(w1_sb[:], w1_d[e, h])
                    w1_tiles[h] = w1_sb
                nc.sync.dma_start(x8_sb[:], x8_d[e])
                nc.sync.dma_start(xa_sb[:, 0:5, :], xa_d[e, :, 0:5, :])
                nc.sync.dma_start(xa_sb[:, 5:DH, :], xa_d[e, :, 5:DH, :])
                nc.sync.dma_start(xb_sb[:, 0:5, :], xb_d[e, :, 0:5, :])
                nc.sync.dma_start(xb_sb[:, 5:DH, :], xb_d[e, :, 5:DH, :])
                for mi in range(1, 4):
                    for h in (mi, 4 + mi):
                        w18_sb = w1p8.tile([P, NF8, P], f8, tag="w18")
                        nc.sync.dma_start(w18_sb[:], w18_d[e, h])
                        w18_tiles[h] = w18_sb
                        w1_sb = w1p.tile([P, DBF * P], f16, tag="w1")
                        nc.sync.dma_start(w1_sb[:], w1_d[e, h])
                        w1_tiles[h] = w1_sb
            w2_sb = w2p.tile([P, HT, D_OUT], f16, tag="w2")
            nc.sync.dma_start(w2_sb[:], w2_d[e])

            hid = hp.tile([P, HT, N], f16, tag="hid")

            if e == 0:
                # Direct GEMM1 + relu -> hiddenT. h0/h1 interleaved in one
                # d-pass so the DMA-paced ramp consumes x at arrival rate.
                pa = [ps1.tile([P, FD], f32, tag="ps1", name=f"pa{i}")
                      for i in range(2)]
                pb = [ps1.tile([P, FD], f32, tag="ps1", name=f"pb{i}")
                      for i in range(2)]
                for dp in range(NPR):
                    s = slice(2 * dp, 2 * dp + 2)
                    for hh in range(2):
                        lhsT8 = w18_tiles[hh][:, s, :]
                        nc.tensor.matmul(pa[hh][:], lhsT8, x8_sb[:, s, 0:FD],
                                         start=(dp == 0), stop=False,
                                         perf_mode=DR)
                        nc.tensor.matmul(pb[hh][:], lhsT8, x8_sb[:, s, FD:N],
                                         start=(dp == 0), stop=False,
                                         perf_mode=DR)
                for d in range(DBF):
                    for hh in range(2):
                        lhsT = w1_tiles[hh][:, bass.ts(d, P)]
                        nc.tensor.matmul(pa[hh][:], lhsT, xd(d, slice(0, FD)),
                                         start=False, stop=(d == DBF - 1))
                        nc.tensor.matmul(pb[hh][:], lhsT, xd(d, slice(FD, N)),
                                         start=False, stop=(d == DBF - 1))
                for hh in range(2):
                    nc.scalar.activation(hid[:, hh, 0:FD], pa[hh][:], relu)
                    nc.scalar.activation(hid[:, hh, FD:N], pb[hh][:], relu)
                for h in range(2, HT):
                    w18_sb = w18_tiles[h]
                    w1_sb = w1_tiles[h]
                    pa1 = ps1.tile([P, FD], f32, tag="ps1")
                    pb1 = ps1.tile([P, FD], f32, tag="ps1")
                    for dp in range(NPR):
                        s = slice(2 * dp, 2 * dp + 2)
                        lhsT8 = w18_sb[:, s, :]
                        nc.tensor.matmul(pa1[:], lhsT8, x8_sb[:, s, 0:FD],
                                         start=(dp == 0), stop=False,
                                         perf_mode=DR)
                        nc.tensor.matmul(pb1[:], lhsT8, x8_sb[:, s, FD:N],
                                         start=(dp == 0), stop=False,
                                         perf_mode=DR)
                    for d in range(DBF):
                        lhsT = w1_sb[:, bass.ts(d, P)]
                        nc.tensor.matmul(pa1[:], lhsT, xd(d, slice(0, FD)),
                                         start=False, stop=(d == DBF - 1))
                        nc.tensor.matmul(pb1[:], lhsT, xd(d, slice(FD, N)),
                                         start=False, stop=(d == DBF - 1))
                    nc.scalar.activation(hid[:, h, 0:FD], pa1[:], relu)
                    nc.scalar.activation(hid[:, h, FD:N], pb1[:], relu)
            else:
                # Strassen-1 GEMM1. X-block sums (shared across mi):
                # X11=xa[:,:,n1] X12=xa[:,:,n2] X21=xb[:,:,n1] X22=xb[:,:,n2]
                n1, n2 = slice(0, FD), slice(FD, N)
                xs1 = xsp.tile([P, DH, FD], f16, tag="xs")
                xs3 = xsp.tile([P, DH, FD], f16, tag="xs")
                xs4 = xsp.tile([P, DH, FD], f16, tag="xs")
                xs6 = xsp.tile([P, DH, FD], f16, tag="xs")
                xs7 = xsp.tile([P, DH, FD], f16, tag="xs")
                stt(xs1[:], xa_sb[:, :, n1], xb_sb[:, :, n2], ADD)
                stt(xs3[:], xa_sb[:, :, n2], xb_sb[:, :, n2], SUB)
                stt(xs4[:], xb_sb[:, :, n1], xa_sb[:, :, n1], SUB)
                stt(xs6[:], xa_sb[:, :, n1], xa_sb[:, :, n2], ADD)
                stt(xs7[:], xb_sb[:, :, n1], xb_sb[:, :, n2], ADD)

                for mi in range(4):
                    wlo = w1_tiles[mi]      # [W11 | W21] chunk
                    whi = w1_tiles[4 + mi]  # [W12 | W22] chunk
                    W11 = wlo[:, 0:HP]
                    W21 = wlo[:, HP: 2 * HP]
                    W12 = whi[:, 0:HP]
                    W22 = whi[:, HP: 2 * HP]
                    ws1 = wsp.tile([P, HP], f16, tag="ws")
                    ws2 = wsp.tile([P, HP], f16, tag="ws")
                    ws5 = wsp.tile([P, HP], f16, tag="ws")
                    ws6 = wsp.tile([P, HP], f16, tag="ws")
                    ws7 = wsp.tile([P, HP], f16, tag="ws")
                    stt(ws1[:], W11, W22, ADD)
                    stt(ws2[:], W12, W22, ADD)
                    stt(ws5[:], W11, W21, ADD)
                    stt(ws6[:], W12, W11, SUB)
                    stt(ws7[:], W21, W22, SUB)

                    def f8quad(hi, ns):
                        p = ps1.tile([P, FD], f32, tag="ps1", name="f8q")
                        for dp in range(NPR):
                            s = slice(2 * dp, 2 * dp + 2)
                            nc.tensor.matmul(p[:], w18_tiles[hi][:, s, :],
                                             x8_sb[:, s, ns],
                                             start=(dp == 0),
                                             stop=(dp == NPR - 1),
                                             perf_mode=DR)
                        return p

                    def product(wt, db, xs):
                        # lhsT = wt[:, (db+d)-th 128-chunk], rhs = xs[:, d, :]
                        p = ps1.tile([P, FD], f32, tag="ps1", name="mprod")
                        for d in range(DH):
                            nc.tensor.matmul(
                                p[:], wt[:, bass.ts(db + d, P)], xs[:, d, :],
                                start=(d == 0), stop=(d == DH - 1))
                        return p

                    def product_raw(wt, db, xraw, ns):
                        p = ps1.tile([P, FD], f32, tag="ps1", name="mraw")
                        for d in range(DH):
                            nc.tensor.matmul(
                                p[:], wt[:, bass.ts(db + d, P)],
                                xraw[:, d, ns],
                                start=(d == 0), stop=(d == DH - 1))
                        return p

                    # psum ring order: F8a,M7,M1,M4,M5,F8b,M3,F8c,M2,F8d,M6
                    ct11 = ctp.tile([P, FD], f32, tag="ct")
                    ct12 = ctp.tile([P, FD], f32, tag="ct")
                    ct21 = ctp.tile([P, FD], f32, tag="ct")
                    ct22 = ctp.tile([P, FD], f32, tag="ct")

                    f8a = f8quad(mi, n1)
                    nc.vector.tensor_scalar_add(ct11[:], f8a[:], 0.0)
                    m7 = product(ws7, 0, xs7)
                    stt(ct11[:], ct11[:], m7[:], ADD)
                    m1 = product(ws1, 0, xs1)
                    stt(ct11[:], ct11[:], m1[:], ADD)
                    nc.vector.tensor_scalar_add(ct22[:], m1[:], 0.0)
                    m4 = product(whi, DH, xs4)
                    stt(ct11[:], ct11[:], m4[:], ADD)
                    m5 = product_raw(ws5, 0, xb_sb, n2)
                    stt(ct11[:], ct11[:], m5[:], SUB)
                    nc.scalar.activation(hid[:, mi, n1], ct11[:], relu)
                    f8b = f8quad(mi, n2)
                    nc.vector.tensor_scalar_add(ct12[:], f8b[:], 0.0)
                    stt(ct12[:], ct12[:], m5[:], ADD)
                    m3 = product(wlo, 0, xs3)
                    stt(ct12[:], ct12[:], m3[:], ADD)
                    nc.scalar.activation(hid[:, mi, n2], ct12[:], relu)
                    f8c = f8quad(4 + mi, n1)
                    nc.vector.tensor_scalar_add(ct21[:], f8c[:], 0.0)
                    m2 = product_raw(ws2, 0, xa_sb, n1)
                    stt(ct21[:], ct21[:], m2[:], ADD)
                    stt(ct22[:], ct22[:], m2[:], SUB)
                    stt(ct21[:], ct21[:], m4[:], ADD)
                    nc.scalar.activation(hid[:, 4 + mi, n1], ct21[:], relu)
                    stt(ct22[:], ct22[:], m3[:], ADD)
                    f8d = f8quad(4 + mi, n2)
                    stt(ct22[:], ct22[:], f8d[:], ADD)
                    m6 = product(ws6, 0, xs6)
                    stt(ct22[:], ct22[:], m6[:], ADD)
                    nc.scalar.activation(hid[:, 4 + mi, n2], ct22[:], relu)

            # GEMM2 + relu, computed transposed (psum [o=128, n=512]).
            o_sb = op.tile([P, 2, NCH, FD], f32, tag="o")
            last_e = e == E_PER - 1
            for nh in range(NCH):
                for oc in range(2):
                    po = ps2.tile([P, FD], f32, tag="ps2")
                    for k in range(HT):
                        nc.tensor.matmul(
                            po[:], w2_sb[:, k, bass.ts(oc, P)],
                            hid[:, k, bass.ds(nh * FD, FD)],
                            start=(k == 0), stop=(k == HT - 1))
                    nc.scalar.activation(o_sb[:, oc, nh, :], po[:], relu)
                    if last_e:
                        nc.scalar.dma_start(
                            out_d[e, bass.ds(oc * P, P), bass.ds(nh * FD, FD)],
                            o_sb[:, oc, nh, :])
            if not last_e:
                for oc in range(2):
                    nc.scalar.dma_start(out_d[e, bass.ds(oc * P, P), :],
                                        o_sb[:, oc])

    nc.compile()
    _CACHE["nc"] = nc
    _CACHE["names"] = ("x8t", "xat", "xbt", "w18t", "w1t", "w2t", "out")
    return nc, _CACHE["names"]


def _prep_inputs(x: np.ndarray, w1: np.ndarray, w2: np.ndarray):
    """Shard across cores + cast + pre-tile so all DMAs are contiguous."""
    xt = (x.astype(_F16).transpose(0, 2, 1)       # [E, D_IN, N]
          .reshape(E, DT, P, N).transpose(0, 2, 1, 3))  # [E, P, DT, N]
    x8t = np.ascontiguousarray(xt[:, :, 0:NF8, :]).astype(_F8)
    xat = np.ascontiguousarray(xt[:, :, NF8: NF8 + DH, :])
    xbt = np.ascontiguousarray(xt[:, :, NF8 + DH:, :])
    w1t = (w1.astype(_F16).reshape(E, DT, P, HT, P)
           .transpose(0, 3, 2, 1, 4))  # [E, HT, P, DT, P]
    w18t = np.ascontiguousarray(
        w1t[:, :, :, 0:NF8, :]).reshape(E, HT, P, NF8 * P).astype(_F8)
    w1bt = np.ascontiguousarray(
        w1t[:, :, :, NF8:, :]).reshape(E, HT, P, DBF * P)
    w2t = np.ascontiguousarray(
        w2.astype(_F16).reshape(E, HT, P, D_OUT).transpose(0, 2, 1, 3))

    in_maps = []
    for c in range(NCORES):
        sl = slice(c * E_PER, (c + 1) * E_PER)
        in_maps.append({"x8t": x8t[sl], "xat": xat[sl], "xbt": xbt[sl],
                        "w18t": w18t[sl], "w1t": w1bt[sl], "w2t": w2t[sl]})
    return in_maps


def run(x, w1, w2, trace=False, **trace_kwargs):
    """Run on 8 cores; returns (full_out, BassKernelResults)."""
    from concourse.bass_utils import run_bass_kernel_spmd

    nc, _ = _build_program()
    in_maps = _prep_inputs(np.asarray(x), np.asarray(w1), np.asarray(w2))
    res = run_bass_kernel_spmd(nc, in_maps, list(range(NCORES)), trace=trace,
                               **trace_kwargs)
    out_t = np.concatenate([res.results[c]["out"] for c in range(NCORES)],
                           axis=0)  # [E, D_OUT, N]
    out = np.ascontiguousarray(out_t.transpose(0, 2, 1))
    return out, res


def _run_in_subprocess(x, w1, w2):
    """Fallback: execute in a fresh interpreter. The NeuronCores are
    occasionally left wedged (NRT_EXEC_UNIT_UNRECOVERABLE on the next
    execute); a fresh process + axon client re-init recovers."""
    import pickle
    import subprocess
    import sys
    import tempfile

    with tempfile.TemporaryDirectory() as td:
        in_p = f"{td}/in.pkl"
        out_p = f"{td}/out.npy"
        with open(in_p, "wb") as f:
            pickle.dump({"x": x, "w1": w1, "w2": w2}, f, protocol=4)
        subprocess.run([sys.executable, __file__, "--subproc", in_p, out_p],
                       check=True, timeout=1200)
        return np.load(out_p)


def kernel(x: np.ndarray, w1: np.ndarray, w2: np.ndarray) -> np.ndarray:
    try:
        out, _ = run(x, w1, w2, trace=False)
        return out
    except Exception:
        pass
    for attempt in range(3):
        try:
            return _run_in_subprocess(x, w1, w2)
        except Exception:
            if attempt == 2:
                raise
    raise RuntimeError("unreachable")


if __name__ == "__main__":
    import pickle
    import sys

    if len(sys.argv) == 4 and sys.argv[1] == "--subproc":
        with open(sys.argv[2], "rb") as f:
            data = pickle.load(f)
        out, _ = run(data["x"], data["w1"], data["w2"], trace=False)
        np.save(sys.argv[3], out)


# revision 13
# speedup vs baseline: 1.1235x; 1.1235x over previous
"""Trainium2 Bass kernel for per-expert MLP (MoE experts, expert-parallel).

Computes out = relu(relu(x @ w1) @ w2) per expert.
  x:  [E=32, N=1024, D_IN=3072] f32
  w1: [E, D_IN, D_H=1024] f32
  w2: [E, D_H, D_OUT=256] f32
  out:[E, N, D_OUT] f32

Sharding: expert dim E=32 split across 8 cores (4 experts/core), no
communication. Host pre-casts and pre-tiles layouts so every DMA is a
plain partition-major copy and no on-chip transposes are needed.

Precision scheme (error budget rel_l2 < 2e-2):
  - GEMM1 K dim (3072 = 24 d-tiles of 128) split: first NF8=6 d-tiles
    use fp8 e4m3 operands via DoubleRow matmuls (2 k-tiles per matmul
    at 2x rate), remaining 18 in fp16 (same PE cost as bf16, 8x less
    quantization noise). Measured rel_l2 ~1.9e-2, dominated by the fp8
    tiles (per-pair err 1.096e-2, sqrt(3)*that total).

Compute scheme:
  - GEMM1 computes hiddenT (h on partitions): lhsT = w1 [d,h] tile,
    rhs = xT [d,n] tile.
  - Experts 1-3: the fp16 portion (K=2304) runs one level of Strassen
    (2x2 over d/h/n halves): 7 products of [1152k x 512h x 512n]
    instead of 8 - 63 matmuls per output quadrant-row vs 72. Operand
    sums on the vector engine (fp16), psum combines on the vector
    engine into SBUF f32 accumulators, relu on scalar. The fp8 portion
    accumulates into per-quadrant psums and joins in the combine.
    PSUM ring (ps1, 6 bufs) alloc order per mi:
    F8a,M7,M1,M4,M5,F8b,M3,F8c,M2,F8d,M6 - verified free-before-reuse.
  - Expert 0 runs the direct path: its ramp is DMA-bound, so compute
    must consume tiles at arrival rate; Strassen needs all operands
    before any product, which would idle the PE ~15us.
  - GEMM2 (K=1024, fp16) direct, computed transposed (psum [o,n]).
"""

import numpy as np
import ml_dtypes

E, N, D_IN, D_H, D_OUT = 32, 1024, 3072, 1024, 256
NCORES = 8
E_PER = E // NCORES  # 4 experts per core
P = 128
DT = D_IN // P  # 24 k-tiles for GEMM1
NF8 = 6         # leading k-tiles in fp8 e4m3 (even: DoubleRow pairs)
NPR = NF8 // 2  # DoubleRow pairs
DBF = DT - NF8  # 18 fp16 k-tiles
DH = DBF // 2   # 9 = d-tiles per Strassen half
HT = D_H // P   # 8 h-tiles
FD = 512        # matmul free dim (one PSUM bank of f32)
NCH = N // FD   # 2 n-chunks in GEMM1

_F16 = np.float16
_F8 = ml_dtypes.float8_e4m3
_CACHE = {}


def _build_program():
    """Build + compile the per-core Bass program (same program on all cores)."""
    if "nc" in _CACHE:
        return _CACHE["nc"], _CACHE["names"]

    from contextlib import ExitStack

    import concourse.bass as bass
    import concourse.tile as tile
    from concourse import bacc, mybir

    f16 = mybir.dt.float16
    f8 = mybir.dt.float8e4
    f32 = mybir.dt.float32
    DR = mybir.MatmulPerfMode.DoubleRow
    ADD = mybir.AluOpType.add
    SUB = mybir.AluOpType.subtract
    MULT = mybir.AluOpType.mult

    nc = bacc.Bacc("TRN2", target_bir_lowering=False, debug=False,
                   enable_asserts=False)

    x8_d = nc.dram_tensor("x8t", [E_PER, P, NF8, N], f8,
                          kind="ExternalInput").ap()
    # x fp16 split in d-halves (Strassen block operands)
    xa_d = nc.dram_tensor("xat", [E_PER, P, NCH, DH, FD], f16,
                          kind="ExternalInput").ap()
    xb_d = nc.dram_tensor("xbt", [E_PER, P, NCH, DH, FD], f16,
                          kind="ExternalInput").ap()
    w18_d = nc.dram_tensor("w18t", [E_PER, HT, P, NF8 * P], f8,
                           kind="ExternalInput").ap()
    w1_d = nc.dram_tensor("w1t", [E_PER, HT, P, DBF * P], f16,
                          kind="ExternalInput").ap()
    w2_d = nc.dram_tensor("w2t", [E_PER, P, HT, D_OUT], f16,
                          kind="ExternalInput").ap()
    out_d = nc.dram_tensor("out", [E_PER, D_OUT, N], f32,
                           kind="ExternalOutput").ap()

    relu = mybir.ActivationFunctionType.Relu
    HP = DH * P  # 1152: fp16 free-dim half of a w1 tile

    with tile.TileContext(nc) as tc, ExitStack() as ctx:
        xp8 = ctx.enter_context(tc.tile_pool(name="x8", bufs=2))
        xp = ctx.enter_context(tc.tile_pool(name="x", bufs=3))
        w1p8 = ctx.enter_context(tc.tile_pool(name="w18", bufs=8))
        w1p = ctx.enter_context(tc.tile_pool(name="w1", bufs=4))
        w2p = ctx.enter_context(tc.tile_pool(name="w2", bufs=2))
        hp = ctx.enter_context(tc.tile_pool(name="hid", bufs=1))
        op = ctx.enter_context(tc.tile_pool(name="o", bufs=1))
        wsp = ctx.enter_context(tc.tile_pool(name="ws", bufs=10))
        xsp = ctx.enter_context(tc.tile_pool(name="xs", bufs=5))
        ctp = ctx.enter_context(tc.tile_pool(name="ct", bufs=6))
        wmp = ctx.enter_context(tc.tile_pool(name="warm", bufs=1))
        ps1 = ctx.enter_context(tc.tile_pool(name="ps1", bufs=6, space="PSUM"))
        ps2 = ctx.enter_context(tc.tile_pool(name="ps2", bufs=2, space="PSUM"))

        def stt(out, in0, in1, op1):
            nc.vector.scalar_tensor_tensor(out, in0, 1.0, in1, MULT, op1)

        def gstt(out, in0, in1, op1):
            nc.gpsimd.scalar_tensor_tensor(out, in0, 1.0, in1, MULT, op1)

        # PE warm-up: dummy matmuls with no data deps fill the initial DMA
        # wait so the HAM clock-gate is at 8/8 (2.4 GHz) when real matmuls
        # start (the un-throttle needs ~3.4us of sustained PE activity).
        NWARM = 18
        warm = wmp.tile([P, FD], f16, tag="warm")
        nc.vector.memset(warm[:], 0.0)
        pw = ps2.tile([P, FD], f32, tag="ps2", name="pw")
        for i in range(NWARM):
            nc.tensor.matmul(pw[:], warm[:, 0:P], warm[:],
                             start=(i == 0), stop=(i == NWARM - 1))

        for e in range(E_PER):
            w18_tiles = [None] * HT
            w1_tiles = [None] * HT
            x8_sb = xp8.tile([P, NF8, N], f8, tag="x8")
            xa_sb = xp.tile([P, NCH, DH, FD], f16, tag="x")
            xb_sb = xp.tile([P, NCH, DH, FD], f16, tag="x")

            def xd(d, nc_i):  # fp16 x d-tile view for n-chunk nc_i
                return (xa_sb[:, nc_i, d, :] if d < DH
                        else xb_sb[:, nc_i, d - DH, :])

            if e == 0:
                # Pace the first-expert ramp (DMA-bandwidth-bound): fp8
                # operands lead (half bytes, feed the leading DoubleRow
                # matmuls), w1 fp16 h0/h1 chunked so the first fp16
                # matmuls only wait on a ~192KB transfer.
                C6 = 6 * P
                w18a = w1p8.tile([P, NF8, P], f8, tag="w18")
                w18b = w1p8.tile([P, NF8, P], f8, tag="w18")
                w18_tiles[0], w18_tiles[1] = w18a, w18b
                w1_sb = w1p.tile([P, DBF * P], f16, tag="w1")
                w1b_sb = w1p.tile([P, DBF * P], f16, tag="w1")
                w1_tiles[0], w1_tiles[1] = w1_sb, w1b_sb
                nc.sync.dma_start(w18a[:], w18_d[e, 0])
                nc.sync.dma_start(w18b[:], w18_d[e, 1])
                nc.sync.dma_start(x8_sb[:, 0:2, :], x8_d[e, :, 0:2, :])
                nc.sync.dma_start(x8_sb[:, 2:4, :], x8_d[e, :, 2:4, :])
                nc.sync.dma_start(x8_sb[:, 4:6, :], x8_d[e, :, 4:6, :])
                nc.sync.dma_start(w1_sb[:, 0:C6], w1_d[e, 0, :, 0:C6])
                nc.sync.dma_start(xa_sb[:, :, 0, :], xa_d[e, :, :, 0, :])
                nc.sync.dma_start(w1b_sb[:, 0:C6], w1_d[e, 1, :, 0:C6])
                nc.sync.dma_start(xa_sb[:, :, 1, :], xa_d[e, :, :, 1, :])
                nc.sync.dma_start(xa_sb[:, :, 2, :], xa_d[e, :, :, 2, :])
                nc.sync.dma_start(w1_sb[:, C6: 2 * C6], w1_d[e, 0, :, C6: 2 * C6])
                nc.sync.dma_start(w1b_sb[:, C6: 2 * C6], w1_d[e, 1, :, C6: 2 * C6])
                nc.sync.dma_start(xa_sb[:, :, 3, :], xa_d[e, :, :, 3, :])
                nc.sync.dma_start(xa_sb[:, :, 4, :], xa_d[e, :, :, 4, :])
                nc.sync.dma_start(w1_sb[:, 2 * C6: DBF * P],
                                  w1_d[e, 0, :, 2 * C6: DBF * P])
                nc.sync.dma_start(w1b_sb[:, 2 * C6: DBF * P],
                                  w1_d[e, 1, :, 2 * C6: DBF * P])
                for d in range(5, DH):
                    nc.sync.dma_start(xa_sb[:, :, d, :], xa_d[e, :, :, d, :])
                for d in range(DH):
                    nc.sync.dma_start(xb_sb[:, :, d, :], xb_d[e, :, :, d, :])
                for h in range(2, HT):
                    w18_sb = w1p8.tile([P, NF8, P], f8, tag="w18")
                    nc.sync.dma_start(w18_sb[:], w18_d[e, h])
                    w18_tiles[h] = w18_sb
                    w1_sb = w1p.tile([P, DBF * P], f16, tag="w1")
                    nc.sync.dma_start(w1_sb[:], w1_d[e, h])
                    w1_tiles[h] = w1_sb
            else:
                # prefetched during previous expert; w1 tiles in paired
                # (h, 4+h) order - the Strassen mi loop consumes them in
                # pairs, and the 4-buf ring frees in the same order.
                for h in (0, 4):
                    w18_sb = w1p8.tile([P, NF8, P], f8, tag="w18")
                    nc.sync.dma_start(w18_sb[:], w18_d[e, h])
                    w18_tiles[h] = w18_sb
                    w1_sb = w1p.tile([P, DBF * P], f16, tag="w1")
                    nc.sync.dma_start(w1_sb[:], w1_d[e, h])
                    w1_tiles[h] = w1_sb
                nc.sync.dma_start(x8_sb[:], x8_d[e])
                nc.sync.dma_start(xa_sb[:, 0], xa_d[e, :, 0])
                nc.sync.dma_start(xa_sb[:, 1], xa_d[e, :, 1])
                nc.sync.dma_start(xb_sb[:, 0], xb_d[e, :, 0])
                nc.sync.dma_start(xb_sb[:, 1], xb_d[e, :, 1])
                for mi in range(1, 4):
                    for h in (mi, 4 + mi):
                        w18_sb = w1p8.tile([P, NF8, P], f8, tag="w18")
                        nc.sync.dma_start(w18_sb[:], w18_d[e, h])
                        w18_tiles[h] = w18_sb
                        w1_sb = w1p.tile([P, DBF * P], f16, tag="w1")
                        nc.sync.dma_start(w1_sb[:], w1_d[e, h])
                        w1_tiles[h] = w1_sb
            w2_sb = w2p.tile([P, HT, D_OUT], f16, tag="w2")
            nc.sync.dma_start(w2_sb[:], w2_d[e])

            hid = hp.tile([P, HT, N], f16, tag="hid")

            if e == 0:
                # Direct GEMM1 + relu -> hiddenT. h0/h1 interleaved in one
                # d-pass so the DMA-paced ramp consumes x at arrival rate.
                pa = [ps1.tile([P, FD], f32, tag="ps1", name=f"pa{i}")
                      for i in range(2)]
                pb = [ps1.tile([P, FD], f32, tag="ps1", name=f"pb{i}")
                      for i in range(2)]
                for dp in range(NPR):
                    s = slice(2 * dp, 2 * dp + 2)
                    for hh in range(2):
                        lhsT8 = w18_tiles[hh][:, s, :]
                        nc.tensor.matmul(pa[hh][:], lhsT8, x8_sb[:, s, 0:FD],
                                         start=(dp == 0), stop=False,
                                         perf_mode=DR)
                        nc.tensor.matmul(pb[hh][:], lhsT8, x8_sb[:, s, FD:N],
                                         start=(dp == 0), stop=False,
                                         perf_mode=DR)
                for d in range(DBF):
                    for hh in range(2):
                        lhsT = w1_tiles[hh][:, bass.ts(d, P)]
                        nc.tensor.matmul(pa[hh][:], lhsT, xd(d, 0),
                                         start=False, stop=(d == DBF - 1))
                        nc.tensor.matmul(pb[hh][:], lhsT, xd(d, 1),
                                         start=False, stop=(d == DBF - 1))
                for hh in range(2):
                    nc.scalar.activation(hid[:, hh, 0:FD], pa[hh][:], relu)
                    nc.scalar.activation(hid[:, hh, FD:N], pb[hh][:], relu)
                for h in range(2, HT):
                    w18_sb = w18_tiles[h]
                    w1_sb = w1_tiles[h]
                    pa1 = ps1.tile([P, FD], f32, tag="ps1")
                    pb1 = ps1.tile([P, FD], f32, tag="ps1")
                    for dp in range(NPR):
                        s = slice(2 * dp, 2 * dp + 2)
                        lhsT8 = w18_sb[:, s, :]
                        nc.tensor.matmul(pa1[:], lhsT8, x8_sb[:, s, 0:FD],
                                         start=(dp == 0), stop=False,
                                         perf_mode=DR)
                        nc.tensor.matmul(pb1[:], lhsT8, x8_sb[:, s, FD:N],
                                         start=(dp == 0), stop=False,
                                         perf_mode=DR)
                    for d in range(DBF):
                        lhsT = w1_sb[:, bass.ts(d, P)]
                        nc.tensor.matmul(pa1[:], lhsT, xd(d, 0),
                                         start=False, stop=(d == DBF - 1))
                        nc.tensor.matmul(pb1[:], lhsT, xd(d, 1),
                                         start=False, stop=(d == DBF - 1))
                    nc.scalar.activation(hid[:, h, 0:FD], pa1[:], relu)
                    nc.scalar.activation(hid[:, h, FD:N], pb1[:], relu)
            else:
                # Strassen-1 GEMM1. X-block sums (shared across mi):
                # X11=xa[:,:,n1] X12=xa[:,:,n2] X21=xb[:,:,n1] X22=xb[:,:,n2]
                n1, n2 = slice(0, FD), slice(FD, N)
                X11, X12 = xa_sb[:, 0], xa_sb[:, 1]
                X21, X22 = xb_sb[:, 0], xb_sb[:, 1]

                def make_xs(a, b, op1):
                    xs = xsp.tile([P, DH, FD], f16, tag="xs", name="xs")
                    stt(xs[:], a, b, op1)
                    return xs

                def make_ws(mi):
                    # order matches first use: M7, M1, M5, M2, M6
                    wlo = w1_tiles[mi]
                    whi = w1_tiles[4 + mi]
                    WB11, WB21 = wlo[:, 0:HP], wlo[:, HP: 2 * HP]
                    WB12, WB22 = whi[:, 0:HP], whi[:, HP: 2 * HP]
                    w = {}
                    for k, i0, i1, op1 in (
                            (7, WB21, WB22, SUB), (1, WB11, WB22, ADD),
                            (5, WB11, WB21, ADD), (2, WB12, WB22, ADD),
                            (6, WB12, WB11, SUB)):
                        t = wsp.tile([P, HP], f16, tag="ws", name="ws")
                        stt(t[:], i0, i1, op1)
                        w[k] = t
                    return w

                # DVE queue in first-use order so products never wait:
                xs7 = make_xs(X21, X22, ADD)
                ws_cur = None
                xs1 = make_xs(X11, X22, ADD)
                ws_cur = make_ws(0)
                xs4 = make_xs(X21, X11, SUB)
                xs3 = make_xs(X12, X22, SUB)
                xs6 = make_xs(X11, X12, ADD)

                for mi in range(4):
                    wlo = w1_tiles[mi]      # [W11 | W21] chunk
                    whi = w1_tiles[4 + mi]  # [W12 | W22] chunk
                    ws1, ws2 = ws_cur[1], ws_cur[2]
                    ws5, ws6, ws7 = ws_cur[5], ws_cur[6], ws_cur[7]

                    def f8quad(hi, ns):
                        p = ps1.tile([P, FD], f32, tag="ps1", name="f8q")
                        for dp in range(NPR):
                            s = slice(2 * dp, 2 * dp + 2)
                            nc.tensor.matmul(p[:], w18_tiles[hi][:, s, :],
                                             x8_sb[:, s, ns],
                                             start=(dp == 0),
                                             stop=(dp == NPR - 1),
                                             perf_mode=DR)
                        return p

                    def product(wt, db, rhs_fn):
                        # lhsT = wt[:, (db+d)-th 128-chunk], rhs = rhs_fn(d)
                        p = ps1.tile([P, FD], f32, tag="ps1", name="mprod")
                        for d in range(DH):
                            nc.tensor.matmul(
                                p[:], wt[:, bass.ts(db + d, P)], rhs_fn(d),
                                start=(d == 0), stop=(d == DH - 1))
                        return p

                    def group(hi, ns, wt, db, rhs_fn):
                        # one psum group: fp8 quad + a single-use M product
                        p = ps1.tile([P, FD], f32, tag="ps1", name="f8m")
                        for dp in range(NPR):
                            s = slice(2 * dp, 2 * dp + 2)
                            nc.tensor.matmul(p[:], w18_tiles[hi][:, s, :],
                                             x8_sb[:, s, ns],
                                             start=(dp == 0), stop=False,
                                             perf_mode=DR)
                        for d in range(DH):
                            nc.tensor.matmul(
                                p[:], wt[:, bass.ts(db + d, P)], rhs_fn(d),
                                start=False, stop=(d == DH - 1))
                        return p

                    # psum ring (6 bufs), allocs per mi:
                    # F8aM7, M1, M4, M5, F8b, M3, F8c, M2, F8dM6
                    ct11 = ctp.tile([P, FD], f32, tag="ct")
                    ct12 = ctp.tile([P, FD], f32, tag="ct")
                    ct21 = ctp.tile([P, FD], f32, tag="ct")
                    ct22 = ctp.tile([P, FD], f32, tag="ct")

                    f8am7 = group(mi, n1, ws7, 0, lambda d: xs7[:, d, :])
                    nc.scalar.copy(ct11[:], f8am7[:])
                    m1 = product(ws1, 0, lambda d: xs1[:, d, :])
                    stt(ct11[:], ct11[:], m1[:], ADD)
                    nc.scalar.copy(ct22[:], m1[:])
                    m4 = product(whi, DH, lambda d: xs4[:, d, :])
                    stt(ct11[:], ct11[:], m4[:], ADD)
                    m5 = product(ws5, 0, lambda d: xb_sb[:, 1, d, :])
                    stt(ct11[:], ct11[:], m5[:], SUB)
                    nc.scalar.activation(hid[:, mi, n1], ct11[:], relu)
                    f8b = f8quad(mi, n2)
                    nc.scalar.copy(ct12[:], f8b[:])
                    stt(ct12[:], ct12[:], m5[:], ADD)
                    m3 = product(wlo, 0, lambda d: xs3[:, d, :])
                    stt(ct12[:], ct12[:], m3[:], ADD)
                    nc.scalar.activation(hid[:, mi, n2], ct12[:], relu)
                    f8c = f8quad(4 + mi, n1)
                    nc.scalar.copy(ct21[:], f8c[:])
                    m2 = product(ws2, 0, lambda d: xa_sb[:, 0, d, :])
                    stt(ct21[:], ct21[:], m2[:], ADD)
                    stt(ct22[:], ct22[:], m2[:], SUB)
                    stt(ct21[:], ct21[:], m4[:], ADD)
                    nc.scalar.activation(hid[:, 4 + mi, n1], ct21[:], relu)
                    stt(ct22[:], ct22[:], m3[:], ADD)
                    f8dm6 = group(4 + mi, n2, ws6, 0, lambda d: xs6[:, d, :])
                    stt(ct22[:], ct22[:], f8dm6[:], ADD)
                    nc.scalar.activation(hid[:, 4 + mi, n2], ct22[:], relu)
                    if mi < 3:
                        ws_cur = make_ws(mi + 1)

            # GEMM2 + relu, computed transposed (psum [o=128, n=512]).
            o_sb = op.tile([P, 2, NCH, FD], f32, tag="o")
            last_e = e == E_PER - 1
            for nh in range(NCH):
                for oc in range(2):
                    po = ps2.tile([P, FD], f32, tag="ps2")
                    for k in range(HT):
                        nc.tensor.matmul(
                            po[:], w2_sb[:, k, bass.ts(oc, P)],
                            hid[:, k, bass.ds(nh * FD, FD)],
                            start=(k == 0), stop=(k == HT - 1))
                    nc.scalar.activation(o_sb[:, oc, nh, :], po[:], relu)
                    if last_e:
                        nc.scalar.dma_start(
                            out_d[e, bass.ds(oc * P, P), bass.ds(nh * FD, FD)],
                            o_sb[:, oc, nh, :])
            if not last_e:
                for oc in range(2):
                    nc.scalar.dma_start(out_d[e, bass.ds(oc * P, P), :],
                                        o_sb[:, oc])

    nc.compile()
    _CACHE["nc"] = nc
    _CACHE["names"] = ("x8t", "xat", "xbt", "w18t", "w1t", "w2t", "out")
    return nc, _CACHE["names"]


def _prep_inputs(x: np.ndarray, w1: np.ndarray, w2: np.ndarray):
    """Shard across cores + cast + pre-tile so all DMAs are contiguous."""
    xt = (x.astype(_F16).transpose(0, 2, 1)       # [E, D_IN, N]
          .reshape(E, DT, P, N).transpose(0, 2, 1, 3))  # [E, P, DT, N]
    x8t = np.ascontiguousarray(xt[:, :, 0:NF8, :]).astype(_F8)
    # n-half-major fp16 halves: [E, P, NCH, DH, FD]
    xat = np.ascontiguousarray(
        xt[:, :, NF8: NF8 + DH, :].reshape(E, P, DH, NCH, FD)
        .transpose(0, 1, 3, 2, 4))
    xbt = np.ascontiguousarray(
        xt[:, :, NF8 + DH:, :].reshape(E, P, DH, NCH, FD)
        .transpose(0, 1, 3, 2, 4))
    w1t = (w1.astype(_F16).reshape(E, DT, P, HT, P)
           .transpose(0, 3, 2, 1, 4))  # [E, HT, P, DT, P]
    w18t = np.ascontiguousarray(
        w1t[:, :, :, 0:NF8, :]).reshape(E, HT, P, NF8 * P).astype(_F8)
    w1bt = np.ascontiguousarray(
        w1t[:, :, :, NF8:, :]).reshape(E, HT, P, DBF * P)
    w2t = np.ascontiguousarray(
        w2.astype(_F16).reshape(E, HT, P, D_OUT).transpose(0, 2, 1, 3))

    in_maps = []
    for c in range(NCORES):
        sl = slice(c * E_PER, (c + 1) * E_PER)
        in_maps.append({"x8t": x8t[sl], "xat": xat[sl], "xbt": xbt[sl],
                        "w18t": w18t[sl], "w1t": w1bt[sl], "w2t": w2t[sl]})
    return in_maps


def run(x, w1, w2, trace=False, **trace_kwargs):
    """Run on 8 cores; returns (full_out, BassKernelResults)."""
    from concourse.bass_utils import run_bass_kernel_spmd

    nc, _ = _build_program()
    in_maps = _prep_inputs(np.asarray(x), np.asarray(w1), np.asarray(w2))
    res = run_bass_kernel_spmd(nc, in_maps, list(range(NCORES)), trace=trace,
                               **trace_kwargs)
    out_t = np.concatenate([res.results[c]["out"] for c in range(NCORES)],
                           axis=0)  # [E, D_OUT, N]
    out = np.ascontiguousarray(out_t.transpose(0, 2, 1))
    return out, res


def _run_in_subprocess(x, w1, w2):
    """Fallback: execute in a fresh interpreter. The NeuronCores are
    occasionally left wedged (NRT_EXEC_UNIT_UNRECOVERABLE on the next
    execute); a fresh process + axon client re-init recovers."""
    import pickle
    import subprocess
    import sys
    import tempfile

    with tempfile.TemporaryDirectory() as td:
        in_p = f"{td}/in.pkl"
        out_p = f"{td}/out.npy"
        with open(in_p, "wb") as f:
            pickle.dump({"x": x, "w1": w1, "w2": w2}, f, protocol=4)
        subprocess.run([sys.executable, __file__, "--subproc", in_p, out_p],
                       check=True, timeout=1200)
        return np.load(out_p)


def kernel(x: np.ndarray, w1: np.ndarray, w2: np.ndarray) -> np.ndarray:
    try:
        out, _ = run(x, w1, w2, trace=False)
        return out
    except Exception:
        pass
    for attempt in range(3):
        try:
            return _run_in_subprocess(x, w1, w2)
        except Exception:
            if attempt == 2:
                raise
    raise RuntimeError("unreachable")


if __name__ == "__main__":
    import pickle
    import sys

    if len(sys.argv) == 4 and sys.argv[1] == "--subproc":
        with open(sys.argv[2], "rb") as f:
            data = pickle.load(f)
        out, _ = run(data["x"], data["w1"], data["w2"], trace=False)
        np.save(sys.argv[3], out)
